# revision 67
# baseline (speedup 1.0000x reference)
"""RoFormer self-attention (LN + fused QKV + interleaved RoPE + SDPA) on 8 trn2 cores.

Sharding: core c -> batch b = c//2, head-group g = c%2 (8 of 16 heads).
Each core computes LN + QKV for its batch (2x-redundant LN within a batch
pair), RoPE, and full non-causal attention for its 8 heads, writing the
[2048, 512] slice out[b, :, 512g:512(g+1)].

Phase 1 (LN + QKV + RoPE): a transpose/LN stage runs LAG chunks ahead of the
QKV stage so the ACT-chain (LN-normalize via Identity-with-scale/bias) and
weight DMAs never stall the PE. QKV weights/activations are bf16 (full PE
rate, halves weight DMA); the projection bias is added on DVE from a
host-replicated bias tile during the RoPE input reads (the matmul therefore
needs no ones-row bias pass). RoPE runs q on DVE, k on Pool, with the final
add emitting bf16 directly; q^T/k^T land via bf16 PE transposes (1.0 cyc/row).

Phase 2 (attention): scores are computed transposed S^T[k, q] (K=64 per
head) into single-bank PSUM tiles; exp'd probabilities P'[k, q] then serve
directly as matmul *stationary* (lhsT) against V[k, 64+1] as the moving
operand, producing ctx[q, 64] accumulated over k-chunks in PSUM with a
ones-column giving the softmax denominator. P'/V are bf16 (bf16 keeps
1 cyc/row even for the 65-wide PV outputs where fp32r would drop to 4).
This keeps PV at full 128x128 utilization and lands the context in [q, d]
layout (no output transposes). Exp throughput is split: ACT takes k-chunk
pairs as one [128, 1024] activation (amortizing its access latency), Pool
computes pow(e^scale, s) on single chunks staged to SBUF by DVE (GPSIMD
cannot read PSUM). Softmax skips the max-subtraction (scores ~ N(0,1)).

Hard-won constraints encoded here: PSUM start/stop act bank-wide (exactly
one accumulation group per bank), GPSIMD cannot access PSUM, and fp32r
matmuls with out-free-size < 256 run at 1/4 rate.
"""

import ml_dtypes
import numpy as np

import concourse.bass as bass
import concourse.mybir as mybir
import concourse.tile as tile
from concourse import bacc
from concourse.bass_utils import run_bass_kernel_spmd

F32 = mybir.dt.float32
F32R = mybir.dt.float32r
BF16 = mybir.dt.bfloat16
AX = mybir.AluOpType
ACT = mybir.ActivationFunctionType

B, S, H = 4, 2048, 1024
NH, HD = 16, 64
LN_EPS = 1e-12
N_CORES = 8
HPC = NH // 2          # 8 heads per core
WCOLS = 3 * HPC * HD   # 1536
TOKCH = S // 128       # 16 token chunks
SCALE = 1.0 / np.sqrt(HD)
EBASE = float(np.exp(SCALE))   # pow(EBASE, s) == exp(s * SCALE)

_CACHE = {}


def _bcast(ap, n, axis=1):
    """Insert a stride-0 broadcast dim of size n at `axis` of an AP."""
    new = [list(p) for p in ap.ap]
    new.insert(axis, [0, n])
    return bass.AP(tensor=ap.tensor, offset=ap.offset, ap=new)


def _build_program():
    nc = bacc.Bacc("TRN2", target_bir_lowering=False)

    hid_d = nc.dram_tensor("hid", [S, H], BF16, kind="ExternalInput")
    w_d = nc.dram_tensor("w", [H, WCOLS], BF16, kind="ExternalInput")
    sin_d = nc.dram_tensor("sintab", [S, HD], F32, kind="ExternalInput")
    cos_d = nc.dram_tensor("costab", [S, HD], F32, kind="ExternalInput")
    idb_d = nc.dram_tensor("identb", [128, 128], BF16, kind="ExternalInput")
    ones_d = nc.dram_tensor("ones", [128, 1], F32R, kind="ExternalInput")
    wb_d = nc.dram_tensor("wbias", [128, WCOLS], F32, kind="ExternalInput")
    out_d = nc.dram_tensor("out", [S, HPC * HD], F32, kind="ExternalOutput")

    with tile.TileContext(nc) as tc:
        with tc.tile_pool(name="const", bufs=1) as const, \
             tc.tile_pool(name="store", bufs=1) as store:
            sin_s = const.tile([128, TOKCH, HD], F32)
            cos_s = const.tile([128, TOKCH, HD], F32)
            idb_s = const.tile([128, 128], BF16)
            nc.sync.dma_start(out=idb_s, in_=idb_d[:, :])
            ones_s = const.tile([128, 1], F32R)
            eps_s = const.tile([128, 1], F32)
            nc.vector.memset(eps_s, LN_EPS)
            expwarm = const.tile([128, 1], F32)
            nc.scalar.activation(expwarm, eps_s, ACT.Exp)
            nhalf_s = const.tile([128, 1], F32)
            nc.vector.memset(nhalf_s, -0.5)
            ebase_s = const.tile([128, 1], F32)
            nc.vector.memset(ebase_s, EBASE)

            # Transposed per-head q/k: head h lives at partitions (h%2)*64,
            # pair index h//2:  [128, 4, TOKCH, 128]  (= [64, tokch*128] per head)
            qT = store.tile([128, HPC // 2, TOKCH, 128], BF16)
            kT = store.tile([128, HPC // 2, TOKCH, 128], BF16)
            # v with appended ones column: [tok, head, 65]
            vA = store.tile([128, TOKCH, HPC, HD + 1], BF16)
            bias_bc = store.tile([128, WCOLS], F32)

            # ---------------- Phase 1: LN + QKV + RoPE + transposes ----------
            with tc.tile_pool(name="wpool", bufs=1) as wpool, \
                 tc.tile_pool(name="p1", bufs=3) as p1, \
                 tc.tile_pool(name="p1h", bufs=3) as p1h, \
                 tc.tile_pool(name="p1n", bufs=3) as p1n, \
                 tc.tile_pool(name="p1s", bufs=6) as p1s, \
                 tc.tile_pool(name="p1T", bufs=5) as p1T, \
                 tc.tile_pool(name="trq", bufs=2, space="PSUM") as trq, \
                 tc.tile_pool(name="qkvpA", bufs=2, space="PSUM") as qkvpA, \
                 tc.tile_pool(name="qkvpB", bufs=2, space="PSUM") as qkvpB:
                w_s = wpool.tile([128, H // 128, WCOLS], BF16)
                w_r = w_d.rearrange("(a p) n -> p a n", p=128)
                # small tables first (id unblocks the PE warmup + transposes),
                # then hidden chunks, then the big weight matrix streams in
                # behind while stage A runs.
                for wu in range(24):
                    ptw = trq.tile([128, 4, 128], BF16, tag="pt4")
                    nc.tensor.transpose(ptw[:, 0, :], idb_s, idb_s)
                ht_tiles = {}
                for tpre in range(2):
                    ht = p1h.tile([128, H], BF16, tag="ht")
                    nc.sync.dma_start(out=ht, in_=hid_d[tpre * 128:(tpre + 1) * 128, :])
                    ht_tiles[tpre] = ht
                nc.sync.dma_start(out=w_s[:, 0, :], in_=w_r[:, 0, :])
                nc.sync.dma_start(out=w_s[:, 1, :], in_=w_r[:, 1, :])

                def rope_block(t, pq):
                    sin_t = sin_s[:, t, :]
                    cos_t = cos_s[:, t, :]
                    bias3 = bias_bc.rearrange("p (n h d) -> p n h d", n=3, d=HD)
                    # v WITHOUT bias (ctx = P(V+bv) = PV + den*bv, so the
                    # V-bias is added after softmax normalization in the
                    # phase-2 tail), plus the ones column for the denominator
                    vv = vA[:, t, :, :]
                    nc.scalar.copy(vv[:, :, 0:HD],
                                   pq[2].rearrange("p (h d) -> p h d", d=HD))
                    nc.gpsimd.tensor_copy(vv[:, :, HD:HD + 1], _bcast(ones_s[:, 0:1], HPC))
                    for nch in range(2):
                        eng = nc.vector if nch == 0 else nc.gpsimd
                        q0 = p1.tile([128, HPC, HD], F32, tag=f"qb{nch}")
                        nc.vector.tensor_tensor(
                            out=q0, in0=pq[nch].rearrange("p (h d) -> p h d", d=HD),
                            in1=bias3[:, nch], op=AX.add)
                        rp = p1.tile([128, HPC, HD], F32, tag="rp")
                        eng.tensor_tensor(out=rp[:, :, 0::2], in0=q0[:, :, 1::2],
                                          in1=_bcast(sin_t[:, 0::2], HPC), op=AX.mult)
                        eng.tensor_tensor(out=rp[:, :, 1::2], in0=q0[:, :, 0::2],
                                          in1=_bcast(sin_t[:, 1::2], HPC), op=AX.mult)
                        qf = p1.tile([128, HPC, HD], F32, tag="qf")
                        eng.tensor_tensor(out=qf, in0=q0, in1=_bcast(cos_t, HPC),
                                          op=AX.mult)
                        qfb = p1.tile([128, HPC, HD], BF16, tag="qfb")
                        eng.tensor_tensor(out=qfb, in0=qf, in1=rp, op=AX.add)

                        dst = qT if nch == 0 else kT
                        pt4 = trq.tile([128, HPC // 2, 128], BF16, tag="pt4")
                        qfv = qfb.rearrange("p h d -> p (h d)")
                        for j in range(HPC // 2):
                            nc.tensor.transpose(pt4[:, j, :],
                                                qfv[:, j * 128:(j + 1) * 128], idb_s)
                        nc.scalar.copy(dst[:, :, t, :], pt4)

                # Stage A (LN + transpose, LAG chunks ahead) feeds stage B
                # (QKV + rope). Decoupling lets PE run transposes while the
                # weight DMAs land, and gives the ACT chain (hn, hT copies)
                # slack so QKV never waits on it.
                LAG = 3
                hT_store = {}
                pending = []  # [(t, pq)] ropes delayed 1 QKV iteration
                for step in range(TOKCH + LAG):
                    if step >= LAG:
                        tq = step - LAG
                        hT = hT_store.pop(tq)
                        # QKV: out[tok, n] accumulated over h-chunks (bias
                        # added during the rope/V ops from bias_bc)
                        pq = []
                        for nch in range(3):
                            pool_ = qkvpA if nch == 0 else qkvpB
                            pp = pool_.tile([128, 512], F32, tag=f"qkv{nch}")
                            for hc in range(H // 128):
                                nc.tensor.matmul(pp, lhsT=hT[:, hc, :],
                                                 rhs=w_s[:, hc, nch * 512:(nch + 1) * 512],
                                                 start=(hc == 0), stop=(hc == H // 128 - 1))
                            pq.append(pp)

                        pending.append((tq, pq))
                        if len(pending) > 1:
                            rope_block(*pending.pop(0))
                    if step < TOKCH:
                        t = step
                        if t + 2 < TOKCH:
                            htn = p1h.tile([128, H], BF16, tag="ht")
                            nc.sync.dma_start(out=htn, in_=hid_d[(t + 2) * 128:(t + 3) * 128, :])
                            ht_tiles[t + 2] = htn
                        if t == 0:
                            for hc in range(2, H // 128):
                                nc.sync.dma_start(out=w_s[:, hc, :], in_=w_r[:, hc, :])
                        if t == 1:
                            # rope tables: not consumed until ~step LAG+1,
                            # so keep them behind the weight burst
                            nc.sync.dma_start(out=sin_s, in_=sin_d.rearrange("(t p) d -> p t d", p=128))
                            nc.sync.dma_start(out=cos_s, in_=cos_d.rearrange("(t p) d -> p t d", p=128))
                            nc.sync.dma_start(out=ones_s, in_=ones_d[:, :])
                            nc.sync.dma_start(out=bias_bc, in_=wb_d[:, :])
                        ht = ht_tiles.pop(t)

                        st6 = p1s.tile([128, 2, 6], F32, tag="st6")
                        for half in range(2):
                            nc.vector.bn_stats(out=st6[:, half, :],
                                               in_=ht[:, half * 512:(half + 1) * 512])
                        mv = p1s.tile([128, 2], F32, tag="mv")
                        nc.vector.bn_aggr(out=mv, in_=st6)
                        vpe = p1s.tile([128, 1], F32, tag="vpe")
                        nc.gpsimd.tensor_scalar(out=vpe, in0=mv[:, 1:2], scalar1=LN_EPS,
                                                scalar2=None, op0=AX.add)
                        rstd = p1s.tile([128, 1], F32, tag="rstd")
                        nc.gpsimd.tensor_tensor(out=rstd, in0=vpe, in1=nhalf_s, op=AX.pow)
                        nmr = p1s.tile([128, 1], F32, tag="nmr")
                        nc.gpsimd.tensor_scalar(out=nmr, in0=rstd, scalar1=mv[:, 0:1],
                                                scalar2=-1.0, op0=AX.mult, op1=AX.mult)

                        # hn = (ht-mu)*rstd, on ACT as identity(ht*rstd - mu*rstd)
                        hn = p1n.tile([128, H], BF16, tag="hn")
                        nc.scalar.activation(hn, ht, ACT.Identity,
                                             bias=nmr[:, 0:1], scale=rstd[:, 0:1])

                        # transpose hn -> hT [hch, tok]
                        hT = p1T.tile([128, H // 128, 128], BF16, tag="hT")
                        for g in range(2):
                            ptg = trq.tile([128, 4, 128], BF16, tag="pt4")
                            for hc in range(4):
                                nc.tensor.transpose(ptg[:, hc, :],
                                                    hn[:, (g * 4 + hc) * 128:(g * 4 + hc + 1) * 128], idb_s)
                            nc.scalar.copy(hT[:, g * 4:(g + 1) * 4, :], ptg)
                        hT_store[t] = hT

                for item in pending:
                    rope_block(*item)

            # ---------------- Phase 2: attention per (head, q-group) ----------
            # Per unit u=(h, qg of 512 q): for kc in 16:
            #   ST:  sp[k 128, q 512] = K^T_kc @ Q        (1 matmul, K=64)
            #   exp: P'[k, q] = exp(SCALE * sp)           (ACT or Pool)
            #   PV:  ctx[q 128, 65] += P'[:, qc]^T @ vA[kc]  (4 matmuls, K=128)
            # then normalize ctx by its ones-column and DMA out. sp tiles are
            # one PSUM bank each -> 6-deep ring hides the ST->exp->PV latency.
            with tc.tile_pool(name="p2", bufs=6) as p2, \
                 tc.tile_pool(name="p2b", bufs=6) as p2b, \
                 tc.tile_pool(name="p2c", bufs=5) as p2c, \
                 tc.tile_pool(name="p2o", bufs=2) as p2o, \
                 tc.tile_pool(name="p2s", bufs=2) as p2s, \
                 tc.tile_pool(name="ctxp", bufs=2, space="PSUM") as ctxp, \
                 tc.tile_pool(name="stpA", bufs=2, space="PSUM") as stpA, \
                 tc.tile_pool(name="stpP", bufs=2, space="PSUM") as stpP:
                QW = 512              # q columns per unit
                QC = QW // 128        # 4 q-chunks per unit
                NQG = S // QW         # 4 q-groups
                units = [(h, qg) for h in range(HPC) for qg in range(NQG)]
                # Exp schedule: ACT handles k-chunk PAIRS as one [128, 1024]
                # activation (amortizes the ~222-cycle access latency); Pool
                # (via a DVE PSUM->SBUF stage, since GPSIMD can't read PSUM)
                # takes single chunks. 10 ACT / 6 Pool per 16.
                SCHED = [("P", 0), ("A", (1, 2)), ("P", 3), ("A", (4, 5)),
                         ("P", 6), ("A", (7, 8)), ("P", 9), ("A", (10, 11)),
                         ("P", 12), ("A", (13, 14)), ("P", 15)]

                def emit_st_into(u, kc, dst):
                    h, qg = u
                    po = (h % 2) * 64
                    pr = h // 2
                    nc.tensor.matmul(
                        dst,
                        lhsT=kT[po:po + 64, pr, kc, :],
                        rhs=qT[po:po + 64, pr, qg * QC:(qg + 1) * QC, :],
                        start=True, stop=True)

                def emit_pv(u, P, kc, ctx):
                    # PSUM start/stop act bank-wide: exactly one start (first
                    # matmul) and one stop (last) for the whole ctx bank.
                    h, qg = u
                    for qc in range(QC):
                        nc.tensor.matmul(
                            ctx[:, qc, 0:HD + 1],
                            lhsT=P[:, qc * 128:(qc + 1) * 128],
                            rhs=vA[:, kc, h, :],
                            start=(kc == 0 and qc == 0),
                            stop=(kc == TOKCH - 1 and qc == QC - 1),
                            skip_group_check=True)

                def emit_exp(kind, sp):
                    # Emitted right after the ST so ACT/DVE/Pool start the
                    # exp chain as soon as the scores land; only the PV
                    # matmuls are delayed by the pending ring.
                    if kind == "A":
                        P = p2b.tile([128, 2, QW], BF16, tag="PA")
                        nc.scalar.activation(P, sp, ACT.Exp, scale=SCALE)
                    else:
                        spc = p2c.tile([128, QW], F32, tag="spc")
                        nc.vector.tensor_copy(spc, sp)
                        P = p2.tile([128, QW], BF16, tag="P")
                        nc.gpsimd.tensor_tensor(out=P, in0=_bcast(ebase_s[:, 0:1], QW),
                                                in1=spc, op=AX.pow)
                    return P

                def emit_consume(u, item, ctx):
                    kind, kcs, P = item
                    if kind == "A":
                        for i, kc in enumerate(kcs):
                            emit_pv(u, P[:, i, :], kc, ctx)
                    else:
                        emit_pv(u, P, kcs, ctx)

                def emit_tail(u, ctx):
                    h, qg = u
                    rec = p2s.tile([128, QC], F32, tag="rec")
                    nc.vector.reciprocal(rec, ctx[:, :, HD])
                    obuf = p2o.tile([128, QC, HD], F32, tag="obuf")
                    nc.vector.tensor_tensor(out=obuf, in0=ctx[:, :, 0:HD],
                                            in1=_bcast(rec, HD, axis=2), op=AX.mult)
                    bv = bias_bc.rearrange("p (n h d) -> p n h d", n=3, d=HD)[:, 2, h, :]
                    nc.vector.tensor_tensor(out=obuf, in0=obuf,
                                            in1=_bcast(bv, QC, axis=1), op=AX.add)
                    dst = out_d[qg * QW:(qg + 1) * QW, h * HD:(h + 1) * HD]
                    nc.sync.dma_start(
                        out=dst.rearrange("(c p) d -> p c d", p=128), in_=obuf)

                # One global stream across units: the next unit's STs
                # interleave with the previous unit's exp/PV drain, so the
                # PE pipeline never empties at unit boundaries.
                # the final unit ends on an ACT pair: the last exp chain
                # sits on the kernel-drain critical path and ACT's chain is
                # ~1us shorter than the staged Pool pow
                SCHED_LAST = [("P", 0), ("A", (1, 2)), ("P", 3), ("A", (4, 5)),
                              ("P", 6), ("A", (7, 8)), ("P", 9), ("A", (10, 11)),
                              ("P", 12), ("P", 13), ("A", (14, 15))]
                stream = [(u, kind, kcs) for u in units
                          for kind, kcs in (SCHED_LAST if u == units[-1] else SCHED)]
                ctx_of = {}
                left_of = {u: len(SCHED) for u in units}

                def consume(entry):
                    u, kind, kcs, P = entry
                    emit_consume(u, (kind, kcs, P), ctx_of[u])
                    left_of[u] -= 1
                    if left_of[u] == 0:
                        emit_tail(u, ctx_of.pop(u))

                pend = []
                for u, kind, kcs in stream:
                    if u not in ctx_of:
                        ctx = ctxp.tile([128, QC, HD + 1], F32, tag="ctx")
                        ctx_of[u] = ctx
                    if kind == "A":
                        sp = stpA.tile([128, 2, QW], F32, tag="stA")
                        for i, kc in enumerate(kcs):
                            emit_st_into(u, kc, sp[:, i, :])
                    else:
                        sp = stpP.tile([128, QW], F32, tag="stP")
                        emit_st_into(u, kcs, sp)
                    pend.append((u, kind, kcs, emit_exp(kind, sp)))
                    if len(pend) > 6:
                        consume(pend.pop(0))
                for entry in pend:
                    consume(entry)

    nc.compile()
    return nc


def _host_inputs(hidden_states, sinusoidal_pos, ln_weight, ln_bias, w_qkv, b_qkv):
    """Build the per-core input maps (all numpy, fp32)."""
    hidden_states = np.ascontiguousarray(hidden_states, dtype=np.float32)
    w_qkv = np.asarray(w_qkv, dtype=np.float32)
    b_qkv = np.asarray(b_qkv, dtype=np.float32)
    ln_weight = np.asarray(ln_weight, dtype=np.float32)
    ln_bias = np.asarray(ln_bias, dtype=np.float32)
    sp = np.asarray(sinusoidal_pos, dtype=np.float32).reshape(S, HD)

    # Fold LayerNorm affine params into the projection.
    w_eff = ln_weight[:, None] * w_qkv          # [H, 3H]
    b_eff = b_qkv + ln_bias @ w_qkv             # [3H]

    sin = sp[:, :HD // 2]
    cos = sp[:, HD // 2:]
    sin_pos = np.repeat(sin, 2, axis=1)          # [S, 64], col 2i = 2i+1 = sin_i
    cos_pos = np.repeat(cos, 2, axis=1)
    sgn = np.ones((1, HD), np.float32)
    sgn[0, 0::2] = -1.0
    sin_signed = (sin_pos * sgn).astype(np.float32)  # col 2i = -sin_i, 2i+1 = sin_i

    ident = np.eye(128, dtype=np.float32)
    ones = np.ones((128, 1), np.float32)

    in_maps = []
    for c in range(N_CORES):
        b = c // 2
        g = c % 2
        cols = np.concatenate([
            np.arange(g * 512, (g + 1) * 512),
            1024 + np.arange(g * 512, (g + 1) * 512),
            2048 + np.arange(g * 512, (g + 1) * 512),
        ])
        in_maps.append({
            "hid": hidden_states[b].astype(ml_dtypes.bfloat16),
            "w": np.ascontiguousarray(w_eff[:, cols]).astype(ml_dtypes.bfloat16),
            "wbias": np.ascontiguousarray(np.broadcast_to(b_eff[cols], (128, len(cols)))),
            "sintab": sin_signed,
            "costab": cos_pos,
            "identb": ident.astype(ml_dtypes.bfloat16),
            "ones": ones,
        })
    return in_maps


def _run(trace=False, **inputs):
    if "nc" not in _CACHE:
        _CACHE["nc"] = _build_program()
    nc = _CACHE["nc"]
    in_maps = _host_inputs(**inputs)
    res = run_bass_kernel_spmd(nc, in_maps, core_ids=list(range(N_CORES)),
                               trace=trace)
    out = np.empty((B, S, H), np.float32)
    for c in range(N_CORES):
        b = c // 2
        g = c % 2
        out[b, :, g * 512:(g + 1) * 512] = res.results[c]["out"]
    return out, res


def kernel(**inputs):
    out, _ = _run(trace=False, **inputs)
    return out


def kernel_traced(**inputs):
    return _run(trace=True, **inputs)


# revision 68
# speedup vs baseline: 1.0007x; 1.0007x over previous
"""RoFormer self-attention (LN + fused QKV + interleaved RoPE + SDPA) on 8 trn2 cores.

Sharding: core c -> batch b = c//2, head-group g = c%2 (8 of 16 heads).
Each core computes LN + QKV for its batch (2x-redundant LN within a batch
pair), RoPE, and full non-causal attention for its 8 heads, writing the
[2048, 512] slice out[b, :, 512g:512(g+1)].

Phase 1 (LN + QKV + RoPE): a transpose/LN stage runs LAG chunks ahead of the
QKV stage so the ACT-chain (LN-normalize via Identity-with-scale/bias) and
weight DMAs never stall the PE. QKV weights/activations are bf16 (full PE
rate, halves weight DMA); the projection bias is added on DVE from a
host-replicated bias tile during the RoPE input reads (the matmul therefore
needs no ones-row bias pass). RoPE runs q on DVE, k on Pool, with the final
add emitting bf16 directly; q^T/k^T land via bf16 PE transposes (1.0 cyc/row).

Phase 2 (attention): scores are computed transposed S^T[k, q] (K=64 per
head) into single-bank PSUM tiles; exp'd probabilities P'[k, q] then serve
directly as matmul *stationary* (lhsT) against V[k, 64+1] as the moving
operand, producing ctx[q, 64] accumulated over k-chunks in PSUM with a
ones-column giving the softmax denominator. P'/V are bf16 (bf16 keeps
1 cyc/row even for the 65-wide PV outputs where fp32r would drop to 4).
This keeps PV at full 128x128 utilization and lands the context in [q, d]
layout (no output transposes). Exp throughput is split: ACT takes k-chunk
pairs as one [128, 1024] activation (amortizing its access latency), Pool
computes pow(e^scale, s) on single chunks staged to SBUF by DVE (GPSIMD
cannot read PSUM). Softmax skips the max-subtraction (scores ~ N(0,1)).

Hard-won constraints encoded here: PSUM start/stop act bank-wide (exactly
one accumulation group per bank), GPSIMD cannot access PSUM, and fp32r
matmuls with out-free-size < 256 run at 1/4 rate.
"""

import ml_dtypes
import numpy as np

import concourse.bass as bass
import concourse.mybir as mybir
import concourse.tile as tile
from concourse import bacc
from concourse.bass_utils import run_bass_kernel_spmd

F32 = mybir.dt.float32
F32R = mybir.dt.float32r
BF16 = mybir.dt.bfloat16
AX = mybir.AluOpType
ACT = mybir.ActivationFunctionType

B, S, H = 4, 2048, 1024
NH, HD = 16, 64
LN_EPS = 1e-12
N_CORES = 8
HPC = NH // 2          # 8 heads per core
WCOLS = 3 * HPC * HD   # 1536
TOKCH = S // 128       # 16 token chunks
SCALE = 1.0 / np.sqrt(HD)
EBASE = float(np.exp(SCALE))   # pow(EBASE, s) == exp(s * SCALE)

_CACHE = {}


def _bcast(ap, n, axis=1):
    """Insert a stride-0 broadcast dim of size n at `axis` of an AP."""
    new = [list(p) for p in ap.ap]
    new.insert(axis, [0, n])
    return bass.AP(tensor=ap.tensor, offset=ap.offset, ap=new)


def _build_program():
    nc = bacc.Bacc("TRN2", target_bir_lowering=False)

    hid_d = nc.dram_tensor("hid", [S, H], BF16, kind="ExternalInput")
    w_d = nc.dram_tensor("w", [H, WCOLS], BF16, kind="ExternalInput")
    sin_d = nc.dram_tensor("sintab", [S, HD], F32, kind="ExternalInput")
    cos_d = nc.dram_tensor("costab", [S, HD], F32, kind="ExternalInput")
    idb_d = nc.dram_tensor("identb", [128, 128], BF16, kind="ExternalInput")
    ones_d = nc.dram_tensor("ones", [128, 1], F32R, kind="ExternalInput")
    wb_d = nc.dram_tensor("wbias", [128, WCOLS], F32, kind="ExternalInput")
    out_d = nc.dram_tensor("out", [S, HPC * HD], F32, kind="ExternalOutput")

    with tile.TileContext(nc) as tc:
        with tc.tile_pool(name="const", bufs=1) as const, \
             tc.tile_pool(name="store", bufs=1) as store:
            sin_s = const.tile([128, TOKCH, HD], F32)
            cos_s = const.tile([128, TOKCH, HD], F32)
            idb_s = const.tile([128, 128], BF16)
            nc.sync.dma_start(out=idb_s, in_=idb_d[:, :])
            ones_s = const.tile([128, 1], F32R)
            eps_s = const.tile([128, 1], F32)
            nc.vector.memset(eps_s, LN_EPS)
            expwarm = const.tile([128, 1], F32)
            nc.scalar.activation(expwarm, eps_s, ACT.Exp)
            nhalf_s = const.tile([128, 1], F32)
            nc.vector.memset(nhalf_s, -0.5)
            ebase_s = const.tile([128, 1], F32)
            nc.vector.memset(ebase_s, EBASE)

            # Transposed per-head q/k: head h lives at partitions (h%2)*64,
            # pair index h//2:  [128, 4, TOKCH, 128]  (= [64, tokch*128] per head)
            qT = store.tile([128, HPC // 2, TOKCH, 128], BF16)
            kT = store.tile([128, HPC // 2, TOKCH, 128], BF16)
            # v with appended ones column: [tok, head, 65]
            vA = store.tile([128, TOKCH, HPC, HD + 1], BF16)
            bias_bc = store.tile([128, WCOLS], F32)

            # ---------------- Phase 1: LN + QKV + RoPE + transposes ----------
            with tc.tile_pool(name="wpool", bufs=1) as wpool, \
                 tc.tile_pool(name="p1", bufs=4) as p1, \
                 tc.tile_pool(name="p1h", bufs=4) as p1h, \
                 tc.tile_pool(name="p1n", bufs=4) as p1n, \
                 tc.tile_pool(name="p1s", bufs=8) as p1s, \
                 tc.tile_pool(name="p1T", bufs=6) as p1T, \
                 tc.tile_pool(name="trq", bufs=2, space="PSUM") as trq, \
                 tc.tile_pool(name="qkvpA", bufs=2, space="PSUM") as qkvpA, \
                 tc.tile_pool(name="qkvpB", bufs=2, space="PSUM") as qkvpB:
                w_s = wpool.tile([128, H // 128, WCOLS], BF16)
                w_r = w_d.rearrange("(a p) n -> p a n", p=128)
                # small tables first (id unblocks the PE warmup + transposes),
                # then hidden chunks, then the big weight matrix streams in
                # behind while stage A runs.
                for wu in range(24):
                    ptw = trq.tile([128, 4, 128], BF16, tag="pt4")
                    nc.tensor.transpose(ptw[:, 0, :], idb_s, idb_s)
                ht_tiles = {}
                for tpre in range(2):
                    ht = p1h.tile([128, H], BF16, tag="ht")
                    nc.sync.dma_start(out=ht, in_=hid_d[tpre * 128:(tpre + 1) * 128, :])
                    ht_tiles[tpre] = ht
                nc.sync.dma_start(out=w_s[:, 0, :], in_=w_r[:, 0, :])
                nc.sync.dma_start(out=w_s[:, 1, :], in_=w_r[:, 1, :])

                def rope_block(t, pq):
                    sin_t = sin_s[:, t, :]
                    cos_t = cos_s[:, t, :]
                    bias3 = bias_bc.rearrange("p (n h d) -> p n h d", n=3, d=HD)
                    # v WITHOUT bias (ctx = P(V+bv) = PV + den*bv, so the
                    # V-bias is added after softmax normalization in the
                    # phase-2 tail), plus the ones column for the denominator
                    vv = vA[:, t, :, :]
                    nc.scalar.copy(vv[:, :, 0:HD],
                                   pq[2].rearrange("p (h d) -> p h d", d=HD))
                    nc.gpsimd.tensor_copy(vv[:, :, HD:HD + 1], _bcast(ones_s[:, 0:1], HPC))
                    for nch in range(2):
                        eng = nc.vector if nch == 0 else nc.gpsimd
                        q0 = p1.tile([128, HPC, HD], F32, tag=f"qb{nch}")
                        nc.vector.tensor_tensor(
                            out=q0, in0=pq[nch].rearrange("p (h d) -> p h d", d=HD),
                            in1=bias3[:, nch], op=AX.add)
                        rp = p1.tile([128, HPC, HD], F32, tag="rp")
                        eng.tensor_tensor(out=rp[:, :, 0::2], in0=q0[:, :, 1::2],
                                          in1=_bcast(sin_t[:, 0::2], HPC), op=AX.mult)
                        eng.tensor_tensor(out=rp[:, :, 1::2], in0=q0[:, :, 0::2],
                                          in1=_bcast(sin_t[:, 1::2], HPC), op=AX.mult)
                        qf = p1.tile([128, HPC, HD], F32, tag="qf")
                        eng.tensor_tensor(out=qf, in0=q0, in1=_bcast(cos_t, HPC),
                                          op=AX.mult)
                        qfb = p1.tile([128, HPC, HD], BF16, tag="qfb")
                        eng.tensor_tensor(out=qfb, in0=qf, in1=rp, op=AX.add)

                        dst = qT if nch == 0 else kT
                        pt4 = trq.tile([128, HPC // 2, 128], BF16, tag="pt4")
                        qfv = qfb.rearrange("p h d -> p (h d)")
                        for j in range(HPC // 2):
                            nc.tensor.transpose(pt4[:, j, :],
                                                qfv[:, j * 128:(j + 1) * 128], idb_s)
                        nc.scalar.copy(dst[:, :, t, :], pt4)

                # Stage A (LN + transpose, LAG chunks ahead) feeds stage B
                # (QKV + rope). Decoupling lets PE run transposes while the
                # weight DMAs land, and gives the ACT chain (hn, hT copies)
                # slack so QKV never waits on it.
                LAG = 3
                hT_store = {}
                pending = []  # [(t, pq)] ropes delayed 1 QKV iteration
                for step in range(TOKCH + LAG):
                    if step >= LAG:
                        tq = step - LAG
                        hT = hT_store.pop(tq)
                        # QKV: out[tok, n] accumulated over h-chunks (bias
                        # added during the rope/V ops from bias_bc)
                        pq = []
                        for nch in range(3):
                            pool_ = qkvpA if nch == 0 else qkvpB
                            pp = pool_.tile([128, 512], F32, tag=f"qkv{nch}")
                            for hc in range(H // 128):
                                nc.tensor.matmul(pp, lhsT=hT[:, hc, :],
                                                 rhs=w_s[:, hc, nch * 512:(nch + 1) * 512],
                                                 start=(hc == 0), stop=(hc == H // 128 - 1))
                            pq.append(pp)

                        pending.append((tq, pq))
                        if len(pending) > 1:
                            rope_block(*pending.pop(0))
                    if step < TOKCH:
                        t = step
                        if t + 2 < TOKCH:
                            htn = p1h.tile([128, H], BF16, tag="ht")
                            nc.sync.dma_start(out=htn, in_=hid_d[(t + 2) * 128:(t + 3) * 128, :])
                            ht_tiles[t + 2] = htn
                        if t == 0:
                            for hc in range(2, H // 128):
                                nc.sync.dma_start(out=w_s[:, hc, :], in_=w_r[:, hc, :])
                        if t == 1:
                            # rope tables: not consumed until ~step LAG+1,
                            # so keep them behind the weight burst
                            nc.sync.dma_start(out=sin_s, in_=sin_d.rearrange("(t p) d -> p t d", p=128))
                            nc.sync.dma_start(out=cos_s, in_=cos_d.rearrange("(t p) d -> p t d", p=128))
                            nc.sync.dma_start(out=ones_s, in_=ones_d[:, :])
                            nc.sync.dma_start(out=bias_bc, in_=wb_d[:, :])
                        ht = ht_tiles.pop(t)

                        st6 = p1s.tile([128, 2, 6], F32, tag="st6")
                        for half in range(2):
                            nc.vector.bn_stats(out=st6[:, half, :],
                                               in_=ht[:, half * 512:(half + 1) * 512])
                        mv = p1s.tile([128, 2], F32, tag="mv")
                        nc.vector.bn_aggr(out=mv, in_=st6)
                        vpe = p1s.tile([128, 1], F32, tag="vpe")
                        nc.gpsimd.tensor_scalar(out=vpe, in0=mv[:, 1:2], scalar1=LN_EPS,
                                                scalar2=None, op0=AX.add)
                        rstd = p1s.tile([128, 1], F32, tag="rstd")
                        nc.gpsimd.tensor_tensor(out=rstd, in0=vpe, in1=nhalf_s, op=AX.pow)
                        nmr = p1s.tile([128, 1], F32, tag="nmr")
                        nc.gpsimd.tensor_scalar(out=nmr, in0=rstd, scalar1=mv[:, 0:1],
                                                scalar2=-1.0, op0=AX.mult, op1=AX.mult)

                        # hn = (ht-mu)*rstd, on ACT as identity(ht*rstd - mu*rstd)
                        hn = p1n.tile([128, H], BF16, tag="hn")
                        nc.scalar.activation(hn, ht, ACT.Identity,
                                             bias=nmr[:, 0:1], scale=rstd[:, 0:1])

                        # transpose hn -> hT [hch, tok]
                        hT = p1T.tile([128, H // 128, 128], BF16, tag="hT")
                        for g in range(2):
                            ptg = trq.tile([128, 4, 128], BF16, tag="pt4")
                            for hc in range(4):
                                nc.tensor.transpose(ptg[:, hc, :],
                                                    hn[:, (g * 4 + hc) * 128:(g * 4 + hc + 1) * 128], idb_s)
                            nc.scalar.copy(hT[:, g * 4:(g + 1) * 4, :], ptg)
                        hT_store[t] = hT

                for item in pending:
                    rope_block(*item)

            # ---------------- Phase 2: attention per (head, q-group) ----------
            # Per unit u=(h, qg of 512 q): for kc in 16:
            #   ST:  sp[k 128, q 512] = K^T_kc @ Q        (1 matmul, K=64)
            #   exp: P'[k, q] = exp(SCALE * sp)           (ACT or Pool)
            #   PV:  ctx[q 128, 65] += P'[:, qc]^T @ vA[kc]  (4 matmuls, K=128)
            # then normalize ctx by its ones-column and DMA out. sp tiles are
            # one PSUM bank each -> 6-deep ring hides the ST->exp->PV latency.
            with tc.tile_pool(name="p2", bufs=6) as p2, \
                 tc.tile_pool(name="p2b", bufs=6) as p2b, \
                 tc.tile_pool(name="p2c", bufs=5) as p2c, \
                 tc.tile_pool(name="p2o", bufs=2) as p2o, \
                 tc.tile_pool(name="p2s", bufs=2) as p2s, \
                 tc.tile_pool(name="ctxp", bufs=2, space="PSUM") as ctxp, \
                 tc.tile_pool(name="stpA", bufs=2, space="PSUM") as stpA, \
                 tc.tile_pool(name="stpP", bufs=2, space="PSUM") as stpP:
                QW = 512              # q columns per unit
                QC = QW // 128        # 4 q-chunks per unit
                NQG = S // QW         # 4 q-groups
                units = [(h, qg) for h in range(HPC) for qg in range(NQG)]
                # Exp schedule: ACT handles k-chunk PAIRS as one [128, 1024]
                # activation (amortizes the ~222-cycle access latency); Pool
                # (via a DVE PSUM->SBUF stage, since GPSIMD can't read PSUM)
                # takes single chunks. 10 ACT / 6 Pool per 16.
                SCHED = [("P", 0), ("A", (1, 2)), ("P", 3), ("A", (4, 5)),
                         ("P", 6), ("A", (7, 8)), ("P", 9), ("A", (10, 11)),
                         ("P", 12), ("A", (13, 14)), ("P", 15)]

                def emit_st_into(u, kc, dst):
                    h, qg = u
                    po = (h % 2) * 64
                    pr = h // 2
                    nc.tensor.matmul(
                        dst,
                        lhsT=kT[po:po + 64, pr, kc, :],
                        rhs=qT[po:po + 64, pr, qg * QC:(qg + 1) * QC, :],
                        start=True, stop=True)

                def emit_pv(u, P, kc, ctx):
                    # PSUM start/stop act bank-wide: exactly one start (first
                    # matmul) and one stop (last) for the whole ctx bank.
                    h, qg = u
                    for qc in range(QC):
                        nc.tensor.matmul(
                            ctx[:, qc, 0:HD + 1],
                            lhsT=P[:, qc * 128:(qc + 1) * 128],
                            rhs=vA[:, kc, h, :],
                            start=(kc == 0 and qc == 0),
                            stop=(kc == TOKCH - 1 and qc == QC - 1),
                            skip_group_check=True)

                def emit_exp(kind, sp):
                    # Emitted right after the ST so ACT/DVE/Pool start the
                    # exp chain as soon as the scores land; only the PV
                    # matmuls are delayed by the pending ring.
                    if kind == "A":
                        P = p2b.tile([128, 2, QW], BF16, tag="PA")
                        nc.scalar.activation(P, sp, ACT.Exp, scale=SCALE)
                    else:
                        spc = p2c.tile([128, QW], F32, tag="spc")
                        nc.vector.tensor_copy(spc, sp)
                        P = p2.tile([128, QW], BF16, tag="P")
                        nc.gpsimd.tensor_tensor(out=P, in0=_bcast(ebase_s[:, 0:1], QW),
                                                in1=spc, op=AX.pow)
                    return P

                def emit_consume(u, item, ctx):
                    kind, kcs, P = item
                    if kind == "A":
                        for i, kc in enumerate(kcs):
                            emit_pv(u, P[:, i, :], kc, ctx)
                    else:
                        emit_pv(u, P, kcs, ctx)

                def emit_tail(u, ctx):
                    h, qg = u
                    rec = p2s.tile([128, QC], F32, tag="rec")
                    nc.vector.reciprocal(rec, ctx[:, :, HD])
                    obuf = p2o.tile([128, QC, HD], F32, tag="obuf")
                    nc.vector.tensor_tensor(out=obuf, in0=ctx[:, :, 0:HD],
                                            in1=_bcast(rec, HD, axis=2), op=AX.mult)
                    bv = bias_bc.rearrange("p (n h d) -> p n h d", n=3, d=HD)[:, 2, h, :]
                    nc.vector.tensor_tensor(out=obuf, in0=obuf,
                                            in1=_bcast(bv, QC, axis=1), op=AX.add)
                    dst = out_d[qg * QW:(qg + 1) * QW, h * HD:(h + 1) * HD]
                    nc.sync.dma_start(
                        out=dst.rearrange("(c p) d -> p c d", p=128), in_=obuf)

                # One global stream across units: the next unit's STs
                # interleave with the previous unit's exp/PV drain, so the
                # PE pipeline never empties at unit boundaries.
                # the final unit ends on an ACT pair: the last exp chain
                # sits on the kernel-drain critical path and ACT's chain is
                # ~1us shorter than the staged Pool pow
                SCHED_LAST = [("P", 0), ("A", (1, 2)), ("P", 3), ("A", (4, 5)),
                              ("P", 6), ("A", (7, 8)), ("P", 9), ("A", (10, 11)),
                              ("P", 12), ("P", 13), ("A", (14, 15))]
                stream = [(u, kind, kcs) for u in units
                          for kind, kcs in (SCHED_LAST if u == units[-1] else SCHED)]
                ctx_of = {}
                left_of = {u: len(SCHED) for u in units}

                def consume(entry):
                    u, kind, kcs, P = entry
                    emit_consume(u, (kind, kcs, P), ctx_of[u])
                    left_of[u] -= 1
                    if left_of[u] == 0:
                        emit_tail(u, ctx_of.pop(u))

                pend = []
                for u, kind, kcs in stream:
                    if u not in ctx_of:
                        ctx = ctxp.tile([128, QC, HD + 1], F32, tag="ctx")
                        ctx_of[u] = ctx
                    if kind == "A":
                        sp = stpA.tile([128, 2, QW], F32, tag="stA")
                        for i, kc in enumerate(kcs):
                            emit_st_into(u, kc, sp[:, i, :])
                    else:
                        sp = stpP.tile([128, QW], F32, tag="stP")
                        emit_st_into(u, kcs, sp)
                    pend.append((u, kind, kcs, emit_exp(kind, sp)))
                    if len(pend) > 6:
                        consume(pend.pop(0))
                for entry in pend:
                    consume(entry)

    nc.compile()
    return nc


def _host_inputs(hidden_states, sinusoidal_pos, ln_weight, ln_bias, w_qkv, b_qkv):
    """Build the per-core input maps (all numpy, fp32)."""
    hidden_states = np.ascontiguousarray(hidden_states, dtype=np.float32)
    w_qkv = np.asarray(w_qkv, dtype=np.float32)
    b_qkv = np.asarray(b_qkv, dtype=np.float32)
    ln_weight = np.asarray(ln_weight, dtype=np.float32)
    ln_bias = np.asarray(ln_bias, dtype=np.float32)
    sp = np.asarray(sinusoidal_pos, dtype=np.float32).reshape(S, HD)

    # Fold LayerNorm affine params into the projection.
    w_eff = ln_weight[:, None] * w_qkv          # [H, 3H]
    b_eff = b_qkv + ln_bias @ w_qkv             # [3H]

    sin = sp[:, :HD // 2]
    cos = sp[:, HD // 2:]
    sin_pos = np.repeat(sin, 2, axis=1)          # [S, 64], col 2i = 2i+1 = sin_i
    cos_pos = np.repeat(cos, 2, axis=1)
    sgn = np.ones((1, HD), np.float32)
    sgn[0, 0::2] = -1.0
    sin_signed = (sin_pos * sgn).astype(np.float32)  # col 2i = -sin_i, 2i+1 = sin_i

    ident = np.eye(128, dtype=np.float32)
    ones = np.ones((128, 1), np.float32)

    in_maps = []
    for c in range(N_CORES):
        b = c // 2
        g = c % 2
        cols = np.concatenate([
            np.arange(g * 512, (g + 1) * 512),
            1024 + np.arange(g * 512, (g + 1) * 512),
            2048 + np.arange(g * 512, (g + 1) * 512),
        ])
        in_maps.append({
            "hid": hidden_states[b].astype(ml_dtypes.bfloat16),
            "w": np.ascontiguousarray(w_eff[:, cols]).astype(ml_dtypes.bfloat16),
            "wbias": np.ascontiguousarray(np.broadcast_to(b_eff[cols], (128, len(cols)))),
            "sintab": sin_signed,
            "costab": cos_pos,
            "identb": ident.astype(ml_dtypes.bfloat16),
            "ones": ones,
        })
    return in_maps


def _run(trace=False, **inputs):
    if "nc" not in _CACHE:
        _CACHE["nc"] = _build_program()
    nc = _CACHE["nc"]
    in_maps = _host_inputs(**inputs)
    res = run_bass_kernel_spmd(nc, in_maps, core_ids=list(range(N_CORES)),
                               trace=trace)
    out = np.empty((B, S, H), np.float32)
    for c in range(N_CORES):
        b = c // 2
        g = c % 2
        out[b, :, g * 512:(g + 1) * 512] = res.results[c]["out"]
    return out, res


def kernel(**inputs):
    out, _ = _run(trace=False, **inputs)
    return out


def kernel_traced(**inputs):
    return _run(trace=True, **inputs)
